# revision 19
# baseline (speedup 1.0000x reference)
"""Trainium2 Bass kernel for LoRA attention prefill (B=4, S=1024, D=4096, H=32).

Sharding: tensor-parallel over heads. Each of the 8 cores computes 4 heads
(512 of the 4096 q/k/v features, column-shard of wq/wk/wv + LoRA B) and a
row-shard of wo, producing a full-shape [T, D] bf16 partial output; partials
are summed on the host.

v2 design (vs the v1 feature-stationary kernel):
  - Stage A is token-stationary: per 128-token block, the x-block is the PE
    stationary operand and the packed [wq|wk|wv|wa] weights stream as the
    moving operand in 512-col matmuls.  RoPE is applied on the free axis
    (feature pairs are strided slices), then one XBAR DMA block-transpose
    per token block produces the [HD, tok] layout the scores matmuls need.
    This kills v1's slow 64-partition scatter DMAs that stalled attention.
  - Attention runs in the natural [query, key] layout: softmax denominators
    come free from the exp activation's accum_out (sum over the free axis),
    reciprocals are [128,1] per query block (v1 burned 106us in 1-partition
    reciprocals), probs are normalized before PV, and the PV operand is
    produced by XBAR block-transposes.  The ones-column denominator matmuls
    and fp32 broadcast matmuls of v1 disappear, and causality is exploited
    at 128-key granularity (25% fewer score/PV columns).
  - Output partials are written in bf16 (half the DMA-out bytes).
"""
import sys
from contextlib import ExitStack

sys.path.insert(0, "/opt/trn_rl_repo")

import numpy as np
import ml_dtypes

import concourse.bass as bass
import concourse.mybir as mybir
import concourse.tile as tile
from concourse import bacc
from concourse.bass_utils import run_bass_kernel_spmd
from concourse.tile import TileContext

B, S, D = 4, 1024, 4096
H, HD = 32, 128
R = 16
LORA_SCALE = 2.0
N_CORES = 8
HPC = H // N_CORES            # heads per core = 4
FPC = HPC * HD                # features per core = 512
T = B * S                     # 4096 tokens
NTB = S // 128                # 128-token blocks per batch = 8
WCOLS = 3 * FPC               # packed weight columns: q | k | v
SCALE = float(1.0 / np.sqrt(HD))
BF = mybir.dt.bfloat16
F32 = mybir.dt.float32


def _bf(a):
    return np.ascontiguousarray(np.asarray(a, np.float32).astype(ml_dtypes.bfloat16))


def _mask_kind(mask):
    mask = np.asarray(mask, np.float32)
    if not mask.any():
        return "zero"
    iu = np.triu_indices(S, k=1)
    il = np.tril_indices(S, k=0)
    if (mask[iu] <= -1e8).all() and (mask[il] == 0).all():
        d = mask[0:128, 0:128]
        for qb in range(1, NTB):
            if not np.array_equal(mask[qb * 128:(qb + 1) * 128,
                                       qb * 128:(qb + 1) * 128], d):
                return "general"
        return "causal"
    return "general"


def _host_prep(x, wq_w, wq_a, wq_b, wk_w, wv_w, wv_a, wv_b, wo_w,
               freqs_cos, freqs_sin, mask):
    x2 = np.asarray(x, np.float32).reshape(T, D)
    xT = _bf(x2.T)                                   # [D, T]

    kind = _mask_kind(mask)
    mask = np.asarray(mask, np.float32)
    sq = np.float32(np.sqrt(HD))
    if kind == "causal":
        maskd = np.ascontiguousarray(mask[0:128, 0:128] * sq)       # [128,128] f32
    elif kind == "general":
        # natural layout [q, k], pre-scaled, bf16, rearranged [128, qb, S]
        mp = (mask * sq).reshape(NTB, 128, S).transpose(1, 0, 2)
        maskd = _bf(np.ascontiguousarray(mp))
    else:
        maskd = None

    cosE = np.asarray(freqs_cos, np.float32).reshape(NTB, 128, 64)
    sinE = np.asarray(freqs_sin, np.float32).reshape(NTB, 128, 64)
    cosE = _bf(cosE.transpose(1, 0, 2))  # [128, tb, 64]
    sinE = _bf(sinE.transpose(1, 0, 2))

    shared = dict(xT=xT, cosE=cosE, sinE=sinE)
    if maskd is not None:
        shared["maskd"] = maskd

    # fold the (linear) LoRA adapters into the base weights:
    #   x@W.T + (x@A.T)@B.T*s == x@(W + s*B@A).T
    wq_eff = (np.asarray(wq_w, np.float32)
              + LORA_SCALE * np.asarray(wq_b, np.float32)
              @ np.asarray(wq_a, np.float32))
    wv_eff = (np.asarray(wv_w, np.float32)
              + LORA_SCALE * np.asarray(wv_b, np.float32)
              @ np.asarray(wv_a, np.float32))
    cores = []
    for c in range(N_CORES):
        sl = slice(c * FPC, (c + 1) * FPC)
        w = np.zeros((D, WCOLS), np.float32)
        w[:, 0:FPC] = wq_eff[sl, :].T
        w[:, FPC:2 * FPC] = np.asarray(wk_w, np.float32)[sl, :].T
        w[:, 2 * FPC:3 * FPC] = wv_eff[sl, :].T
        cores.append(dict(
            wpk=_bf(w),
            woT=_bf(np.asarray(wo_w, np.float32)[:, sl].T),
        ))
    return shared, cores, kind


def _build_program(kind):
    nc = bacc.Bacc("TRN2", num_devices=N_CORES)
    causal = kind == "causal"

    xT = nc.dram_tensor("xT", [D, T], BF, kind="ExternalInput").ap()
    wpk = nc.dram_tensor("wpk", [D, WCOLS], BF, kind="ExternalInput").ap()
    woT = nc.dram_tensor("woT", [FPC, D], BF, kind="ExternalInput").ap()
    cosE = nc.dram_tensor("cosE", [128, NTB, 64], BF, kind="ExternalInput").ap()
    sinE = nc.dram_tensor("sinE", [128, NTB, 64], BF, kind="ExternalInput").ap()
    if kind == "causal":
        maskd = nc.dram_tensor("maskd", [128, 128], F32, kind="ExternalInput").ap()
    elif kind == "general":
        maskd = nc.dram_tensor("maskd", [128, NTB, S], BF, kind="ExternalInput").ap()
    y = nc.dram_tensor("y", [T, D], BF, kind="ExternalOutput").ap()

    # general-mask variant carries a 16KB/partition mask: shrink elsewhere
    xbufs = 2 if kind == "general" else 3
    prbufs = 3
    tmpbufs = 2

    with TileContext(nc) as tc, ExitStack() as ctx:
        wpool = ctx.enter_context(tc.tile_pool(name="wpool", bufs=1))
        xpool = ctx.enter_context(tc.tile_pool(name="xpool", bufs=xbufs))
        natp = ctx.enter_context(tc.tile_pool(name="natp", bufs=2))
        tmpp = ctx.enter_context(tc.tile_pool(name="tmpp", bufs=tmpbufs))
        qkp = ctx.enter_context(tc.tile_pool(name="qkp", bufs=1))
        vp = ctx.enter_context(tc.tile_pool(name="vp", bufs=1))
        otp = ctx.enter_context(tc.tile_pool(name="otp", bufs=1))
        prp = ctx.enter_context(tc.tile_pool(name="prp", bufs=prbufs))
        ptp0 = ctx.enter_context(tc.tile_pool(name="ptp0", bufs=2))
        pt1bufs = 3 if kind == "zero" else 2
        ptp1 = ctx.enter_context(tc.tile_pool(name="ptp1", bufs=pt1bufs))
        denp = ctx.enter_context(tc.tile_pool(name="denp", bufs=2))
        wop = ctx.enter_context(tc.tile_pool(name="wop", bufs=2))
        outp = ctx.enter_context(tc.tile_pool(name="outp", bufs=3))
        psm = ctx.enter_context(tc.tile_pool(name="psm", bufs=6, space="PSUM"))
        psa = ctx.enter_context(tc.tile_pool(name="psa", bufs=2, space="PSUM"))

        # ---- resident tensors ----
        w_sb = wpool.tile([128, 32, WCOLS], BF, tag="wpk")
        nc.sync.dma_start(w_sb[:], wpk.rearrange("(o p) f -> p o f", p=128))
        cos_sb = wpool.tile([128, NTB, 64], BF, tag="cos")
        nc.sync.dma_start(cos_sb[:], cosE[:])
        sin_sb = wpool.tile([128, NTB, 64], BF, tag="sin")
        nc.sync.dma_start(sin_sb[:], sinE[:])
        if kind == "causal":
            mask_sb = wpool.tile([128, 128], F32, tag="mask")
            nc.sync.dma_start(mask_sb[:], maskd[:])
        elif kind == "general":
            mask_sb = wpool.tile([128, NTB, S], BF, tag="mask")
            nc.sync.dma_start(mask_sb[:], maskd[:])

        def load_x(b, tb):
            x_sb = xpool.tile([128, 32, 128], BF, tag="x")
            t0 = b * S + tb * 128
            nc.sync.dma_start(
                x_sb[:],
                xT.rearrange("(o p) t -> p o t", p=128)[:, :, t0:t0 + 128])
            return x_sb

        def rope(ps, nat_sb, tb):
            # ps [128 tok, 512 feat] f32 -> nat_sb [128, 512] bf16, rotated
            pv = ps[:].rearrange("p (h k two) -> p h k two", h=HPC, two=2)
            ne = nat_sb[:].rearrange("p (h k two) -> p h k two", h=HPC, two=2)
            cbc = cos_sb[:, tb, None, :].to_broadcast((128, HPC, 64))
            sbc = sin_sb[:, tb, None, :].to_broadcast((128, HPC, 64))
            q_e = pv[:, :, :, 0]
            q_o = pv[:, :, :, 1]
            t1 = tmpp.tile([128, HPC, 64], F32, tag="t")
            nc.vector.tensor_tensor(t1[:], q_e, cbc, mybir.AluOpType.mult)
            t2 = tmpp.tile([128, HPC, 64], F32, tag="t")
            nc.vector.tensor_tensor(t2[:], q_o, sbc, mybir.AluOpType.mult)
            nc.vector.tensor_tensor(ne[:, :, :, 0], t1[:], t2[:],
                                    mybir.AluOpType.subtract)
            t3 = tmpp.tile([128, HPC, 64], F32, tag="t")
            nc.vector.tensor_tensor(t3[:], q_e, sbc, mybir.AluOpType.mult)
            t4 = tmpp.tile([128, HPC, 64], F32, tag="t")
            nc.vector.tensor_tensor(t4[:], q_o, cbc, mybir.AluOpType.mult)
            nc.vector.tensor_tensor(ne[:, :, :, 1], t3[:], t4[:],
                                    mybir.AluOpType.add)

        for b in range(B):
            QT_sb = qkp.tile([128, HPC, S], BF, tag="QT")
            KT_sb = qkp.tile([128, HPC, S], BF, tag="KT")
            V_sb = vp.tile([128, NTB, FPC], BF, tag="V")
            OT_sb = otp.tile([128, HPC, S], BF, tag="OT")

            # ---------------- attention helpers ----------------
            def scores_block(l, qh):
                # probsT tile: [key-in-block, kb, local query]
                if causal and qh == 0:
                    pt = ptp0.tile([128, 4, 512], BF, tag="pt0")
                else:
                    pt = ptp1.tile([128, NTB, 512], BF, tag="pt1")
                den = denp.tile([128, 4, 2], F32, tag="den")
                rec = denp.tile([128, 4], F32, tag="rec")
                for j in range(4):
                    qb = qh * 4 + j
                    q0 = qb * 128
                    kmax = (qb + 1) * 128 if causal else S
                    nk = kmax // 128
                    probs = prp.tile([128, 1024], BF, tag="probs")
                    pieces = []
                    for p0 in range(0, kmax, 512):
                        pw = min(512, kmax - p0)
                        ps_sc = psm.tile([128, 512], F32, tag="psm")
                        nc.tensor.matmul(
                            ps_sc[:, 0:pw], QT_sb[:, l, q0:q0 + 128],
                            KT_sb[:, l, p0:p0 + pw], start=True, stop=True)
                        pieces.append((p0, pw, ps_sc))
                    if causal:
                        # triangular mask on the diagonal 128-key block
                        p0, pw, ps_sc = pieces[-1]
                        off = qb * 128 - p0
                        nc.vector.tensor_add(
                            ps_sc[:, off:off + 128], ps_sc[:, off:off + 128],
                            mask_sb[:])
                    elif kind == "general":
                        for p0, pw, ps_sc in pieces:
                            nc.vector.tensor_add(
                                ps_sc[:, 0:pw], ps_sc[:, 0:pw],
                                mask_sb[:, qb, p0:p0 + pw])
                    for pi, (p0, pw, ps_sc) in enumerate(pieces):
                        nc.scalar.activation(
                            probs[:, p0:p0 + pw], ps_sc[:, 0:pw],
                            mybir.ActivationFunctionType.Exp, scale=SCALE,
                            accum_out=den[:, j, pi:pi + 1])
                    if len(pieces) > 1:
                        nc.vector.tensor_add(den[:, j, 0:1], den[:, j, 0:1],
                                             den[:, j, 1:2])
                    nc.vector.reciprocal(rec[:, j:j + 1], den[:, j, 0:1])
                    nc.vector.tensor_scalar_mul(
                        probs[:, 0:kmax], probs[:, 0:kmax], rec[:, j:j + 1])
                    nc.sync.dma_start_transpose(
                        pt[:, 0:nk, j * 128:(j + 1) * 128], probs[:, 0:kmax])
                return pt

            def pv_block(l, qh, pt):
                ps_ot = psa.tile([128, 512], F32, tag="psa")
                nfull = qh * 4 if causal else NTB
                for kb in range(nfull):
                    nc.tensor.matmul(
                        ps_ot[:], V_sb[:, kb, l * 128:(l + 1) * 128],
                        pt[:, kb, :], start=(kb == 0),
                        stop=(not causal and kb == NTB - 1))
                if causal:
                    # diagonal 512x512 zone: per (qb, kb) valid 128-col pieces
                    for j in range(4):
                        qb = qh * 4 + j
                        for kb in range(nfull, qb + 1):
                            nc.tensor.matmul(
                                ps_ot[:, j * 128:(j + 1) * 128],
                                V_sb[:, kb, l * 128:(l + 1) * 128],
                                pt[:, kb, j * 128:(j + 1) * 128],
                                start=(nfull == 0 and kb == 0),
                                stop=(kb == qb))
                nc.vector.tensor_copy(OT_sb[:, l, qh * 512:(qh + 1) * 512],
                                      ps_ot[:])

            # ---------------- stage A with qh0 attention interleaved ----
            pts = {}
            x_tiles = [load_x(b, 0), load_x(b, 1)]
            for tb in range(NTB):
                x_sb = x_tiles[tb]
                if tb + 2 < NTB:
                    x_tiles.append(load_x(b, tb + 2))
                q_ps = psm.tile([128, 512], F32, tag="psm")
                k_ps = psm.tile([128, 512], F32, tag="psm")
                v_ps = psm.tile([128, 512], F32, tag="psm")
                for d in range(32):
                    xb = x_sb[:, d, :]
                    nc.tensor.matmul(q_ps[:], xb, w_sb[:, d, 0:FPC],
                                     start=(d == 0), stop=(d == 31))
                    nc.tensor.matmul(k_ps[:], xb, w_sb[:, d, FPC:2 * FPC],
                                     start=(d == 0), stop=(d == 31))
                    nc.tensor.matmul(v_ps[:], xb, w_sb[:, d, 2 * FPC:3 * FPC],
                                     start=(d == 0), stop=(d == 31))

                # RoPE + transpose for Q and K; V copies straight out
                qnat = natp.tile([128, 512], BF, tag="nat")
                rope(q_ps, qnat, tb)
                knat = natp.tile([128, 512], BF, tag="nat")
                rope(k_ps, knat, tb)
                nc.scalar.copy(V_sb[:, tb, :], v_ps[:])
                nc.scalar.dma_start_transpose(
                    QT_sb[:, :, tb * 128:(tb + 1) * 128], qnat[:])
                nc.scalar.dma_start_transpose(
                    KT_sb[:, :, tb * 128:(tb + 1) * 128], knat[:])

                # qh0 attention stages ride inside tb 4..7 (keys tb 0..3 and
                # the stage's own QT block are ready by then).  Issued after
                # this tb's psum readers so pool-FIFO aliasing stays ordered.
                if tb >= 4:
                    l = tb - 4
                    pts[(l, 0)] = scores_block(l, 0)
                    if l >= 1:
                        pv_block(l - 1, 0, pts.pop((l - 1, 0)))

            # ---------------- qh1 attention ----------------
            pts[(0, 1)] = scores_block(0, 1)
            pv_block(3, 0, pts.pop((3, 0)))
            for l in range(1, HPC):
                pts[(l, 1)] = scores_block(l, 1)
                pv_block(l - 1, 1, pts.pop((l - 1, 1)))
            pv_block(3, 1, pts.pop((3, 1)))

            # ---------------- stage C ----------------
            for nt in range(8):
                wo_sb = wop.tile([128, HPC, 512], BF, tag="wo")
                eng = nc.gpsimd if nt == 0 else nc.scalar
                eng.dma_start(
                    wo_sb[:],
                    woT.rearrange("(o p) n -> p o n",
                                  p=128)[:, :, nt * 512:(nt + 1) * 512])
                for tp in range(4):
                    o_sb = outp.tile([128, 2, 512], BF, tag="o")
                    for half in range(2):
                        tb = tp * 2 + half
                        ps_o = psm.tile([128, 512], F32, tag="psm")
                        for k in range(HPC):
                            nc.tensor.matmul(
                                ps_o[:], OT_sb[:, k, tb * 128:(tb + 1) * 128],
                                wo_sb[:, k, :], start=(k == 0), stop=(k == 3))
                        if half == 0:
                            nc.scalar.copy(o_sb[:, half, :], ps_o[:])
                        else:
                            nc.vector.tensor_copy(o_sb[:, half, :], ps_o[:])
                    t0 = b * S + tp * 256
                    eng = nc.sync if tp % 2 == 0 else nc.gpsimd
                    eng.dma_start(
                        y[t0:t0 + 256, nt * 512:(nt + 1) * 512].rearrange(
                            "(i p) n -> p i n", p=128), o_sb[:])

    nc.compile()
    return nc


_CACHE = {}


def _get_program(kind):
    if kind not in _CACHE:
        _CACHE[kind] = _build_program(kind)
    return _CACHE[kind]


def kernel(x, wq_w, wq_a, wq_b, wk_w, wv_w, wv_a, wv_b, wo_w,
           freqs_cos, freqs_sin, mask, start_pos=0, _trace=False):
    assert int(np.asarray(start_pos)) == 0
    shared, cores, kind = _host_prep(
        x, wq_w, wq_a, wq_b, wk_w, wv_w, wv_a, wv_b, wo_w,
        freqs_cos, freqs_sin, mask)
    nc = _get_program(kind)
    in_maps = []
    for c in range(N_CORES):
        m = dict(shared)
        m.update(cores[c])
        in_maps.append(m)
    res = run_bass_kernel_spmd(nc, in_maps, list(range(N_CORES)),
                               trace=_trace)
    kernel._last_results = res
    acc = np.zeros((T, D), np.float32)
    for c in range(N_CORES):
        acc += np.asarray(res.results[c]["y"], np.float32)
    out = acc.reshape(B, S, D)
    return out.astype(np.asarray(x).dtype, copy=False)


# revision 20
# speedup vs baseline: 1.1783x; 1.1783x over previous
"""Trainium2 Bass kernel for LoRA attention prefill (B=4, S=1024, D=4096, H=32).

Sharding: tensor-parallel over heads. Each of the 8 cores computes 4 heads
(512 of the 4096 q/k/v features, column-shard of wq/wk/wv + LoRA B) and a
row-shard of wo, producing a full-shape [T, D] bf16 partial output; partials
are summed on the host.

v2 design (vs the v1 feature-stationary kernel):
  - Stage A is token-stationary: per 128-token block, the x-block is the PE
    stationary operand and the packed [wq|wk|wv|wa] weights stream as the
    moving operand in 512-col matmuls.  RoPE is applied on the free axis
    (feature pairs are strided slices), then one XBAR DMA block-transpose
    per token block produces the [HD, tok] layout the scores matmuls need.
    This kills v1's slow 64-partition scatter DMAs that stalled attention.
  - Attention runs in the natural [query, key] layout: softmax denominators
    come free from the exp activation's accum_out (sum over the free axis),
    reciprocals are [128,1] per query block (v1 burned 106us in 1-partition
    reciprocals), probs are normalized before PV, and the PV operand is
    produced by XBAR block-transposes.  The ones-column denominator matmuls
    and fp32 broadcast matmuls of v1 disappear, and causality is exploited
    at 128-key granularity (25% fewer score/PV columns).
  - Output partials are written in bf16 (half the DMA-out bytes).
"""
import sys
from contextlib import ExitStack

sys.path.insert(0, "/opt/trn_rl_repo")

import numpy as np
import ml_dtypes

import concourse.bass as bass
import concourse.mybir as mybir
import concourse.tile as tile
from concourse import bacc
from concourse.bass_utils import run_bass_kernel_spmd
from concourse.tile import TileContext

B, S, D = 4, 1024, 4096
H, HD = 32, 128
R = 16
LORA_SCALE = 2.0
N_CORES = 8
HPC = H // N_CORES            # heads per core = 4
FPC = HPC * HD                # features per core = 512
T = B * S                     # 4096 tokens
NTB = S // 128                # 128-token blocks per batch = 8
WCOLS = 3 * FPC               # packed weight columns: q | k | v
SCALE = float(1.0 / np.sqrt(HD))
BF = mybir.dt.bfloat16
F32 = mybir.dt.float32


def _bf(a):
    return np.ascontiguousarray(np.asarray(a, np.float32).astype(ml_dtypes.bfloat16))


def _mask_kind(mask):
    mask = np.asarray(mask, np.float32)
    if not mask.any():
        return "zero"
    iu = np.triu_indices(S, k=1)
    il = np.tril_indices(S, k=0)
    if (mask[iu] <= -1e8).all() and (mask[il] == 0).all():
        d = mask[0:128, 0:128]
        for qb in range(1, NTB):
            if not np.array_equal(mask[qb * 128:(qb + 1) * 128,
                                       qb * 128:(qb + 1) * 128], d):
                return "general"
        return "causal"
    return "general"


def _host_prep(x, wq_w, wq_a, wq_b, wk_w, wv_w, wv_a, wv_b, wo_w,
               freqs_cos, freqs_sin, mask):
    x2 = np.asarray(x, np.float32).reshape(T, D)
    xT = _bf(x2.T)                                   # [D, T]

    kind = _mask_kind(mask)
    mask = np.asarray(mask, np.float32)
    sq = np.float32(np.sqrt(HD))
    if kind == "causal":
        maskd = np.ascontiguousarray(mask[0:128, 0:128] * sq)       # [128,128] f32
    elif kind == "general":
        # natural layout [q, k], pre-scaled, bf16, rearranged [128, qb, S]
        mp = (mask * sq).reshape(NTB, 128, S).transpose(1, 0, 2)
        maskd = _bf(np.ascontiguousarray(mp))
    else:
        maskd = None

    cosE = np.asarray(freqs_cos, np.float32).reshape(NTB, 128, 64)
    sinE = np.asarray(freqs_sin, np.float32).reshape(NTB, 128, 64)
    cosE = _bf(cosE.transpose(1, 0, 2))  # [128, tb, 64]
    sinE = _bf(sinE.transpose(1, 0, 2))

    shared = dict(xT=xT, cosE=cosE, sinE=sinE)
    if maskd is not None:
        shared["maskd"] = maskd

    # fold the (linear) LoRA adapters into the base weights:
    #   x@W.T + (x@A.T)@B.T*s == x@(W + s*B@A).T
    wq_eff = (np.asarray(wq_w, np.float32)
              + LORA_SCALE * np.asarray(wq_b, np.float32)
              @ np.asarray(wq_a, np.float32))
    wv_eff = (np.asarray(wv_w, np.float32)
              + LORA_SCALE * np.asarray(wv_b, np.float32)
              @ np.asarray(wv_a, np.float32))
    cores = []
    for c in range(N_CORES):
        sl = slice(c * FPC, (c + 1) * FPC)
        w = np.zeros((D, WCOLS), np.float32)
        w[:, 0:FPC] = wq_eff[sl, :].T
        w[:, FPC:2 * FPC] = np.asarray(wk_w, np.float32)[sl, :].T
        w[:, 2 * FPC:3 * FPC] = wv_eff[sl, :].T
        cores.append(dict(
            wpk=_bf(w),
            woT=_bf(np.asarray(wo_w, np.float32)[:, sl].T),
        ))
    return shared, cores, kind


def _build_program(kind):
    nc = bacc.Bacc("TRN2", num_devices=N_CORES)
    causal = kind == "causal"

    xT = nc.dram_tensor("xT", [D, T], BF, kind="ExternalInput").ap()
    wpk = nc.dram_tensor("wpk", [D, WCOLS], BF, kind="ExternalInput").ap()
    woT = nc.dram_tensor("woT", [FPC, D], BF, kind="ExternalInput").ap()
    cosE = nc.dram_tensor("cosE", [128, NTB, 64], BF, kind="ExternalInput").ap()
    sinE = nc.dram_tensor("sinE", [128, NTB, 64], BF, kind="ExternalInput").ap()
    if kind == "causal":
        maskd = nc.dram_tensor("maskd", [128, 128], F32, kind="ExternalInput").ap()
    elif kind == "general":
        maskd = nc.dram_tensor("maskd", [128, NTB, S], BF, kind="ExternalInput").ap()
    y = nc.dram_tensor("y", [T, D], BF, kind="ExternalOutput").ap()

    # general-mask variant carries a 16KB/partition mask: shrink elsewhere
    xbufs = 2 if kind == "general" else 3
    prbufs = 3
    tmpbufs = 2

    with TileContext(nc) as tc, ExitStack() as ctx:
        wpool = ctx.enter_context(tc.tile_pool(name="wpool", bufs=1))
        xpool = ctx.enter_context(tc.tile_pool(name="xpool", bufs=xbufs))
        natp = ctx.enter_context(tc.tile_pool(name="natp", bufs=2))
        tmpp = ctx.enter_context(tc.tile_pool(name="tmpp", bufs=tmpbufs))
        qkp = ctx.enter_context(tc.tile_pool(name="qkp", bufs=1))
        vp = ctx.enter_context(tc.tile_pool(name="vp", bufs=1))
        otp = ctx.enter_context(tc.tile_pool(name="otp", bufs=1))
        prp = ctx.enter_context(tc.tile_pool(name="prp", bufs=prbufs))
        ptp0 = ctx.enter_context(tc.tile_pool(name="ptp0", bufs=2))
        pt1bufs = 3 if kind == "zero" else 2
        ptp1 = ctx.enter_context(tc.tile_pool(name="ptp1", bufs=pt1bufs))
        denp = ctx.enter_context(tc.tile_pool(name="denp", bufs=2))
        wop = ctx.enter_context(tc.tile_pool(name="wop", bufs=2))
        outp = ctx.enter_context(tc.tile_pool(name="outp", bufs=3))
        psm = ctx.enter_context(tc.tile_pool(name="psm", bufs=7, space="PSUM"))
        psa = ctx.enter_context(tc.tile_pool(name="psa", bufs=1, space="PSUM"))

        # ---- resident tensors ----
        w_sb = wpool.tile([128, 32, WCOLS], BF, tag="wpk")
        wpk_r = wpk.rearrange("(o p) f -> p o f", p=128)
        for ci, eng in enumerate((nc.sync, nc.scalar, nc.gpsimd, nc.scalar)):
            eng.dma_start(w_sb[:, ci * 8:(ci + 1) * 8, :],
                          wpk_r[:, ci * 8:(ci + 1) * 8, :])
        cos_sb = wpool.tile([128, NTB, 64], BF, tag="cos")
        nc.gpsimd.dma_start(cos_sb[:], cosE[:])
        sin_sb = wpool.tile([128, NTB, 64], BF, tag="sin")
        nc.gpsimd.dma_start(sin_sb[:], sinE[:])
        zrow = wpool.tile([1, 512], BF, tag="zrow")
        nc.gpsimd.memset(zrow[:], 0.0)
        if kind == "causal":
            mask_sb = wpool.tile([128, 128], F32, tag="mask")
            nc.sync.dma_start(mask_sb[:], maskd[:])
        elif kind == "general":
            mask_sb = wpool.tile([128, NTB, S], BF, tag="mask")
            nc.sync.dma_start(mask_sb[:], maskd[:])

        def load_x(b, tb):
            x_sb = xpool.tile([128, 32, 128], BF, tag="x")
            t0 = b * S + tb * 128
            nc.sync.dma_start(
                x_sb[:],
                xT.rearrange("(o p) t -> p o t", p=128)[:, :, t0:t0 + 128])
            return x_sb

        def rope(ps, nat_sb, tb):
            # ps [128 tok, 512 feat] f32 -> nat_sb [128, 512] bf16, rotated
            pv = ps[:].rearrange("p (h k two) -> p h k two", h=HPC, two=2)
            ne = nat_sb[:].rearrange("p (h k two) -> p h k two", h=HPC, two=2)
            cbc = cos_sb[:, tb, None, :].to_broadcast((128, HPC, 64))
            sbc = sin_sb[:, tb, None, :].to_broadcast((128, HPC, 64))
            q_e = pv[:, :, :, 0]
            q_o = pv[:, :, :, 1]
            t1 = tmpp.tile([128, HPC, 64], F32, tag="t")
            nc.vector.tensor_tensor(t1[:], q_e, cbc, mybir.AluOpType.mult)
            t2 = tmpp.tile([128, HPC, 64], F32, tag="t")
            nc.vector.tensor_tensor(t2[:], q_o, sbc, mybir.AluOpType.mult)
            nc.vector.tensor_tensor(ne[:, :, :, 0], t1[:], t2[:],
                                    mybir.AluOpType.subtract)
            t3 = tmpp.tile([128, HPC, 64], F32, tag="t")
            nc.vector.tensor_tensor(t3[:], q_e, sbc, mybir.AluOpType.mult)
            t4 = tmpp.tile([128, HPC, 64], F32, tag="t")
            nc.vector.tensor_tensor(t4[:], q_o, cbc, mybir.AluOpType.mult)
            nc.vector.tensor_tensor(ne[:, :, :, 1], t3[:], t4[:],
                                    mybir.AluOpType.add)

        for b in range(B):
            QT_sb = qkp.tile([128, HPC, S], BF, tag="QT")
            KT_sb = qkp.tile([128, HPC, S], BF, tag="KT")
            V_sb = vp.tile([128, NTB, FPC], BF, tag="V")
            OT_sb = otp.tile([128, HPC, S], BF, tag="OT")

            # ---------------- attention helpers ----------------
            def scores_block(l, qh):
                # probsT tile: [key-in-block, kb, local query]
                if causal and qh == 0:
                    pt = ptp0.tile([128, 4, 512], BF, tag="pt0")
                else:
                    pt = ptp1.tile([128, NTB, 512], BF, tag="pt1")
                den = denp.tile([128, 4, 2], F32, tag="den")
                rec = denp.tile([128, 4], F32, tag="rec")
                for j in range(4):
                    qb = qh * 4 + j
                    q0 = qb * 128
                    kmax = (qb + 1) * 128 if causal else S
                    nk = kmax // 128
                    probs = prp.tile([128, 1024], BF, tag="probs")
                    pieces = []
                    for p0 in range(0, kmax, 512):
                        pw = min(512, kmax - p0)
                        ps_sc = psm.tile([128, 512], F32, tag="psm")
                        nc.tensor.matmul(
                            ps_sc[:, 0:pw], QT_sb[:, l, q0:q0 + 128],
                            KT_sb[:, l, p0:p0 + pw], start=True, stop=True)
                        pieces.append((p0, pw, ps_sc))
                    if causal:
                        # triangular mask on the diagonal 128-key block
                        p0, pw, ps_sc = pieces[-1]
                        off = qb * 128 - p0
                        nc.vector.tensor_add(
                            ps_sc[:, off:off + 128], ps_sc[:, off:off + 128],
                            mask_sb[:])
                    elif kind == "general":
                        for p0, pw, ps_sc in pieces:
                            nc.vector.tensor_add(
                                ps_sc[:, 0:pw], ps_sc[:, 0:pw],
                                mask_sb[:, qb, p0:p0 + pw])
                    for pi, (p0, pw, ps_sc) in enumerate(pieces):
                        nc.scalar.activation(
                            probs[:, p0:p0 + pw], ps_sc[:, 0:pw],
                            mybir.ActivationFunctionType.Exp, scale=SCALE,
                            accum_out=den[:, j, pi:pi + 1])
                    if len(pieces) > 1:
                        nc.vector.tensor_add(den[:, j, 0:1], den[:, j, 0:1],
                                             den[:, j, 1:2])
                    nc.vector.reciprocal(rec[:, j:j + 1], den[:, j, 0:1])
                    nc.vector.tensor_scalar_mul(
                        probs[:, 0:kmax], probs[:, 0:kmax], rec[:, j:j + 1])
                    nc.sync.dma_start_transpose(
                        pt[:, 0:nk, j * 128:(j + 1) * 128], probs[:, 0:kmax])
                return pt

            def pv_block(l, qh, pt):
                ps_ot = psa.tile([128, 512], F32, tag="psa")
                nfull = qh * 4 if causal else NTB
                for kb in range(nfull):
                    nc.tensor.matmul(
                        ps_ot[:], V_sb[:, kb, l * 128:(l + 1) * 128],
                        pt[:, kb, :], start=(kb == 0),
                        stop=(not causal and kb == NTB - 1))
                if causal:
                    # diagonal 512x512 zone: key-block kb contributes to all
                    # query cols >= its own, so one widening matmul per kb;
                    # a K=1 zero matmul closes the accumulation group.
                    for i in range(4):
                        kb = nfull + i
                        nc.tensor.matmul(
                            ps_ot[:, i * 128:512],
                            V_sb[:, kb, l * 128:(l + 1) * 128],
                            pt[:, kb, i * 128:512],
                            start=(nfull == 0 and i == 0), stop=False)
                    nc.tensor.matmul(ps_ot[:], zrow[0:1, 0:128],
                                     zrow[0:1, :], start=False, stop=True)
                nc.vector.tensor_copy(OT_sb[:, l, qh * 512:(qh + 1) * 512],
                                      ps_ot[:])

            # ---------------- stage A with qh0 attention interleaved ----
            pts = {}
            x_tiles = [load_x(b, 0), load_x(b, 1)]
            for tb in range(NTB):
                x_sb = x_tiles[tb]
                if tb + 2 < NTB:
                    x_tiles.append(load_x(b, tb + 2))
                q_ps = psm.tile([128, 512], F32, tag="psm")
                k_ps = psm.tile([128, 512], F32, tag="psm")
                v_ps = psm.tile([128, 512], F32, tag="psm")
                for d in range(32):
                    xb = x_sb[:, d, :]
                    nc.tensor.matmul(q_ps[:], xb, w_sb[:, d, 0:FPC],
                                     start=(d == 0), stop=(d == 31))
                    nc.tensor.matmul(k_ps[:], xb, w_sb[:, d, FPC:2 * FPC],
                                     start=(d == 0), stop=(d == 31))
                    nc.tensor.matmul(v_ps[:], xb, w_sb[:, d, 2 * FPC:3 * FPC],
                                     start=(d == 0), stop=(d == 31))

                # RoPE + transpose for Q and K; V copies straight out
                qnat = natp.tile([128, 512], BF, tag="nat")
                rope(q_ps, qnat, tb)
                knat = natp.tile([128, 512], BF, tag="nat")
                rope(k_ps, knat, tb)
                nc.scalar.copy(V_sb[:, tb, :], v_ps[:])
                nc.scalar.dma_start_transpose(
                    QT_sb[:, :, tb * 128:(tb + 1) * 128], qnat[:])
                nc.scalar.dma_start_transpose(
                    KT_sb[:, :, tb * 128:(tb + 1) * 128], knat[:])

                # qh0 attention stages ride inside tb 4..7 (keys tb 0..3 and
                # the stage's own QT block are ready by then).  Issued after
                # this tb's psum readers so pool-FIFO aliasing stays ordered.
                if tb >= 4:
                    l = tb - 4
                    pts[(l, 0)] = scores_block(l, 0)
                    if l >= 1:
                        pv_block(l - 1, 0, pts.pop((l - 1, 0)))

            # ---------------- qh1 attention ----------------
            pts[(0, 1)] = scores_block(0, 1)
            pv_block(3, 0, pts.pop((3, 0)))
            for l in range(1, HPC):
                pts[(l, 1)] = scores_block(l, 1)
                pv_block(l - 1, 1, pts.pop((l - 1, 1)))
            pv_block(3, 1, pts.pop((3, 1)))

            # ---------------- stage C ----------------
            for nt in range(8):
                wo_sb = wop.tile([128, HPC, 512], BF, tag="wo")
                eng = nc.gpsimd if nt == 0 else nc.scalar
                eng.dma_start(
                    wo_sb[:],
                    woT.rearrange("(o p) n -> p o n",
                                  p=128)[:, :, nt * 512:(nt + 1) * 512])
                for tp in range(4):
                    o_sb = outp.tile([128, 2, 512], BF, tag="o")
                    for half in range(2):
                        tb = tp * 2 + half
                        ps_o = psm.tile([128, 512], F32, tag="psm")
                        for k in range(HPC):
                            nc.tensor.matmul(
                                ps_o[:], OT_sb[:, k, tb * 128:(tb + 1) * 128],
                                wo_sb[:, k, :], start=(k == 0), stop=(k == 3))
                        if half == 0:
                            nc.scalar.copy(o_sb[:, half, :], ps_o[:])
                        else:
                            nc.vector.tensor_copy(o_sb[:, half, :], ps_o[:])
                    t0 = b * S + tp * 256
                    eng = nc.sync if tp % 2 == 0 else nc.gpsimd
                    eng.dma_start(
                        y[t0:t0 + 256, nt * 512:(nt + 1) * 512].rearrange(
                            "(i p) n -> p i n", p=128), o_sb[:])

    nc.compile()
    return nc


_CACHE = {}


def _get_program(kind):
    if kind not in _CACHE:
        _CACHE[kind] = _build_program(kind)
    return _CACHE[kind]


def kernel(x, wq_w, wq_a, wq_b, wk_w, wv_w, wv_a, wv_b, wo_w,
           freqs_cos, freqs_sin, mask, start_pos=0, _trace=False):
    assert int(np.asarray(start_pos)) == 0
    shared, cores, kind = _host_prep(
        x, wq_w, wq_a, wq_b, wk_w, wv_w, wv_a, wv_b, wo_w,
        freqs_cos, freqs_sin, mask)
    nc = _get_program(kind)
    in_maps = []
    for c in range(N_CORES):
        m = dict(shared)
        m.update(cores[c])
        in_maps.append(m)
    res = run_bass_kernel_spmd(nc, in_maps, list(range(N_CORES)),
                               trace=_trace)
    kernel._last_results = res
    acc = np.zeros((T, D), np.float32)
    for c in range(N_CORES):
        acc += np.asarray(res.results[c]["y"], np.float32)
    out = acc.reshape(B, S, D)
    return out.astype(np.asarray(x).dtype, copy=False)
